# revision 63
# baseline (speedup 1.0000x reference)
"""Trainium2 Bass kernel for the 2-layer LSTM 'Conductor' module.

Reference computation (fp32, B=1024, H=1024, STEPS=4):
    h0=c0=h1=c1=z; each step: x=z -> LSTM0 -> LSTM1 -> collect h1
    out[b,s,:] = h1(s) @ W_lin^T + b_lin            -> [1024, 4, 1024] f32

Strategy: data-parallel over 8 NeuronCores (128 batch rows each); weights
replicated. v2: the three recurrent weight matrices (W_hh0, W_ih1, W_hh1)
are stored in fp8 E3M4 scaled by 256 (values in +-8, inside e3m4's normal
range), which halves their HBM bytes vs bf16 and lets ALL weights stay
SBUF-resident (no per-step re-streaming; DMA drops ~59MB -> ~19MB/core).
Matmuls run mixed bf16-stationary x fp8-moving at the bf16 rate; the 1/256
descale is folded into the transpose identity (hT tiles hold h/256, exact
in bf16) and W_lin is kept bf16 x256 (exact), so all scales cancel with
zero extra instructions. Cell state c stays fp32 on-chip. Simulated rel
err 5.2e-3 vs the fp32 reference (e4m3 would be 8.4e-3; fp8 activations
would be 2.6e-2 and are NOT used).

Per-core kernel, gates laid out [batch=128 partitions, 4H free]:
  - activations are the stationary lhsT (= x^T chunks, bf16); weights are
    the moving rhs (W^T, blocked by gate-group then k-chunk); N=512 streams
  - per gate group, PSUM [128,1024] accumulates both products, VectorE adds
    the bias/const tile into PSUM, ScalarE applies sigmoid/tanh PSUM->SBUF,
    VectorE does the cell update, PE transpose-matmuls rebuild h^T
  - software pipelining: step s+1's layer-0 matmuls are emitted between
    layer-1's matmuls and its elementwise tail, hiding the EW latency
  - z-projections (z@W_ih0+b0, step-0 gates z@(W_ih0+W_hh0)+b0, and step-0
    layer-1's z@W_hh1+b1) are host-precomputed input transforms (~2% FLOPs)
  - startup fill (~19MB) rides both HWDGE rings in consumption order;
    after the fill there is no DMA except the 4 output tiles

With weights resident the kernel is TensorE-bound (~162us of bf16 matmul
at peak); baseline (bf16 streamed) measured 237-249us.
"""
import sys

sys.path.insert(0, '/opt/trn_rl_repo')

import numpy as np
import concourse.bass as bass
import concourse.mybir as mybir
import concourse.tile as tile
from concourse.vector_clock import ScopedClock
from concourse.bass_utils import run_bass_kernel_spmd

B, H, STEPS, CORES = 1024, 1024, 4, 8
BC = B // CORES          # batch rows per core
KC = H // 128            # 8 contraction chunks
G = 4                    # gate groups (i, f, g, o), 1024 cols each
F32 = mybir.dt.float32
BF16 = mybir.dt.bfloat16
F8 = mybir.dt.float8e4   # E4M3 (TRN flavor, max +-240): DoubleRow-eligible
E3 = mybir.dt.float8e3   # E3M4: gate consts ride fp8 to halve fill bytes
NP_BF16 = mybir.dt.np(BF16)
NP_F8 = mybir.dt.np(F8)
NP_E3 = mybir.dt.np(E3)
WS = 512.0               # weight scale (W*512 in +-16, e4m3 normal range)
HS = 16.0                # hT scale (h*16 in e4m3; ident = I*16)
S = WS * HS              # matmul product scale; consts pre-scaled by S,
ISCALE = 1.0 / S         # descaled inside the gate activations
LINS = 256.0             # W_lin*256 bf16; h1T16 = h/256 (exact powers of 2)
DR = mybir.MatmulPerfMode.DoubleRow
SIG = mybir.ActivationFunctionType.Sigmoid
TANH = mybir.ActivationFunctionType.Tanh


def _drain_and_barrier_split(self, tick_clock, wait_clock):
    # Same as TileContext._drain_and_barrier, but the final drain's sem waits
    # are split onto single-wait SP nops: walrus's Drain codegen
    # (TPB_CTRL_NO_STRUCT setupSyncWait) rejects >2 waits on one instruction.
    nc = self.nc
    probe = nc.sync.nop(nofuse=True)
    wait_clock.add_sem_waits(probe.ins, ScopedClock({None: tick_clock.global_clock}))
    waits = []
    if probe.ins.sync_info and probe.ins.sync_info.on_wait:
        waits = list(probe.ins.sync_info.on_wait)
    probe.ins.sync_info = mybir.SyncInfo(on_wait=waits[:1], on_update=[])
    for w in waits[1:]:
        n = nc.sync.nop(nofuse=True)
        n.ins.sync_info = mybir.SyncInfo(on_wait=[w], on_update=[])
    nc.sync.drain()
    nc.all_engine_barrier()
    popped = nc._tile_sem_poison_stack.pop()
    assert popped is self._sem_poison
    nc.clear_and_free_semaphores(list(self.sems.allocated().values()))
    nc.all_engine_barrier()


tile.TileContext._drain_and_barrier = _drain_and_barrier_split


def _split_sync_waits(nc, max_waits=1):
    """walrus's setupSyncWait rejects instructions carrying >1 sem wait.

    Move excess waits onto same-engine nops inserted immediately before the
    offending instruction (program order on the engine preserves semantics).
    """
    n_split = 0
    for f in nc.m.functions:
        for blk in f.blocks:
            il = blk.instructions
            idx = 0
            while idx < len(il):
                inst = il[idx]
                si = inst.sync_info
                if si is not None and si.on_wait and len(si.on_wait) > max_waits:
                    waits = list(si.on_wait)
                    keep, extra = waits[-max_waits:], waits[:-max_waits]
                    chunks = [extra[i:i + max_waits] for i in range(0, len(extra), max_waits)]
                    for ci, chunk in enumerate(chunks):
                        n = mybir.InstNoOp(name=f"{inst.name}-wsplit{ci}", ins=[], outs=[])
                        n.engine = inst.engine
                        n.sync_info = mybir.SyncInfo(on_wait=list(chunk), on_update=[])
                        il.insert(idx, n)
                        idx += 1
                        n_split += 1
                    inst.sync_info = mybir.SyncInfo(
                        on_wait=keep,
                        on_update=list(si.on_update) if si.on_update else [],
                    )
                idx += 1
    return n_split


def _emit(nc, tc, t):
    """Emit the per-core program. t: dict of DRAM APs.

    Software-pipelined emission (PE program order per iteration s):
        [tr h0(s)] [L1(s) mms] [L0(s+1) mms] [tr h1(s)] [linear(s) mms]
    so next-step L0 matmuls cover the L1 elementwise chain and L1 matmuls
    cover the L0 elementwise chain. Step 0's L0 gates come fully
    host-precomputed (constA); step 0's L1 recurrent part too (constB).

    All weights are SBUF-resident; the startup fill is issued up front on
    both HWDGE rings in per-ring consumption order.
    """
    mm = nc.tensor.matmul
    sdma = nc.scalar.dma_start   # ACT-ring DMA
    wdma = nc.sync.dma_start     # SP-ring DMA
    gdma = nc.gpsimd.dma_start   # software-DGE queue (third stream)
    with (
        tc.tile_pool(name="res", bufs=1) as rpool,
        tc.tile_pool(name="cpool", bufs=1) as cpool,
        tc.tile_pool(name="state", bufs=1) as spool,
        tc.tile_pool(name="hT", bufs=2) as htpool,
        tc.tile_pool(name="ab", bufs=6) as abpool,
        tc.tile_pool(name="af", bufs=4) as afpool,
        tc.tile_pool(name="hp", bufs=2) as hpool,
        tc.tile_pool(name="op", bufs=1) as opool,
        tc.tile_pool(name="gpsum", bufs=3, space="PSUM") as gpsum,
        tc.tile_pool(name="tpsum", bufs=2, space="PSUM") as tpsum,
    ):
        # -- resident tiles --------------------------------------------------
        # Weights and constA are split into per-gate tiles: transfers into
        # ONE tile serialize end-to-end on the completion semaphore (queue
        # depth 1), so distinct tiles are required for back-to-back streaming.
        # identity variants: transposes use I*16 (hT = h*16); const openers
        # fold each const tensor's storage scale back to the psum domain
        ident = rpool.tile([128, 128], BF16, name="ident")       # = I*16
        idS = rpool.tile([128, 128], BF16, name="idS")           # = I*4096
        idB = rpool.tile([128, 128], BF16, name="idB")           # = I*128
        idb = rpool.tile([128, 128], BF16, name="idb")           # = I/256
        # gate consts in e3m4: constA/constB/const0 store value*2 (+-10,
        # inside e3m4 normal range); b1b stores value*64 (biases are +-0.06,
        # subnormal at x2); blinb stores b_lin*256
        caT = [cpool.tile([128, 1024], E3, name=f"constA{g}") for g in range(G)]
        cBT = [cpool.tile([128, 1024], E3, name=f"constB{g}") for g in range(G)]
        c0T = [cpool.tile([128, 1024], E3, name=f"const0{g}") for g in range(G)]
        b1T = [cpool.tile([128, 1024], E3, name=f"b1b{g}") for g in range(G)]
        blinb = rpool.tile([128, 1024], E3, name="blinb")
        # cell state in bf16: EW ops become all-16-bit, eligible for the
        # DVE 2x_1p double-rate mode (simulated rel err 1.08e-2, unchanged)
        c0 = spool.tile([128, 1024], BF16, name="c0")
        c1 = spool.tile([128, 1024], BF16, name="c1")
        # wih1/whh0 as per-gate k-HALF tiles (finer arrival granularity in
        # the DMA-paced head); whh1 as per-gate tiles on the gpsimd queue
        whh0 = [rpool.tile([128, 4, 1024], F8, name=f"whh0g{g}h{h}")
                for g in range(G) for h in range(2)]
        wih1 = [rpool.tile([128, 4, 1024], F8, name=f"wih1g{g}h{h}")
                for g in range(G) for h in range(2)]
        whh1 = [rpool.tile([128, KC, 1024], F8, name=f"whh1g{g}") for g in range(G)]
        wlinh = [rpool.tile([128, 4096], BF16, name=f"wlinh{h}") for h in range(2)]

        def whalf(name, g, h):
            return t[name][:, g * 8192 + h * 4096:g * 8192 + (h + 1) * 4096]

        def wsl(w, g, kk, lo):
            """[128,2,512] DR rhs slice from per-gate k-half weight tiles."""
            return w[g * 2 + kk // 4][:, kk % 4:kk % 4 + 2, lo:lo + 512]

        # -- startup fill ----------------------------------------------------
        # Every transfer targets its own tile (same-tile transfers serialize
        # end-to-end), and each ring's issue order is its arrival order,
        # matched to first-consumption order. The head is DMA-paced: L1(s0)
        # and L0(s1) consume weights faster than the rings deliver, so the
        # constants are interleaved between weight chunks exactly when their
        # PSUM-opener matmuls need them. whh1 (needed last) rides the gpsimd
        # software-DGE queue as a third stream.
        wdma(ident[:], t["ident"][:])
        wdma(caT[0][:], t["constA"][:, 0:1024])
        wdma(c0[:], t["zb16"][:])
        wdma(idS[:], t["idS"][:])
        wdma(idB[:], t["idB"][:])
        wdma(idb[:], t["idb"][:])
        nc.vector.tensor_copy(c1[:], c0[:])
        wdma(wih1[0][:], whalf("w_ih1", 0, 0))
        wdma(wih1[1][:], whalf("w_ih1", 0, 1))
        wdma(wih1[4][:], whalf("w_ih1", 2, 0))
        wdma(wih1[5][:], whalf("w_ih1", 2, 1))
        wdma(c0T[0][:], t["const0"][:, 0:1024])
        wdma(whh0[0][:], whalf("w_hh0", 0, 0))
        wdma(whh0[1][:], whalf("w_hh0", 0, 1))
        wdma(c0T[2][:], t["const0"][:, 2048:3072])
        wdma(whh0[4][:], whalf("w_hh0", 2, 0))
        wdma(whh0[5][:], whalf("w_hh0", 2, 1))
        wdma(b1T[0][:], t["b1b"][:, 0:1024])
        wdma(whh1[0][:], t["w_hh1"][:, 0:8192])
        wdma(blinb[:], t["blinb"][:])
        wdma(wlinh[0][:], t["w_lin"][:, 0:4096])
        wdma(b1T[2][:], t["b1b"][:, 2048:3072])
        wdma(whh1[2][:], t["w_hh1"][:, 16384:24576])
        sdma(caT[1][:], t["constA"][:, 1024:2048])
        sdma(caT[2][:], t["constA"][:, 2048:3072])
        sdma(caT[3][:], t["constA"][:, 3072:4096])
        sdma(cBT[0][:], t["constB"][:, 0:1024])

        def gate_psum(name, cadd, opener):
            """Fresh gate PSUM, opened (start=True) by an identity-matmul
            that accumulates the pre-scaled const/bias tile into the bank on
            the PE itself (~0.43us/gate); the opener identity's scale maps
            the const's storage scale to the psum domain. This removes the
            VectorE const-add and its cross-engine latency from every gate's
            matmul->activation chain; later gate matmuls use start=False."""
            ps = gpsum.tile([128, 1024], F32, tag="gates", name=name)
            mm(ps[:, 0:512], opener[:], cadd[:, 0:512], start=True, stop=False)
            mm(ps[:, 512:1024], opener[:], cadd[:, 512:1024],
               start=True, stop=False)
            return ps

        # PE warmup: dummy matmuls so HAM unthrottles (4/8 -> 8/8) before
        # real work; they also bridge PE-idle pockets in the fill window so
        # HAM doesn't re-throttle.
        wup = tpsum.tile([128, 512], F32, tag="tr", name="warmup_ps")

        def warm(n):
            for _ in range(n):
                mm(wup[:, 0:128], ident[:], ident[:], start=True, stop=True)

        warm_n = [0]

        def warm_at(n):
            """Mid-program warmup burst: allocates a fresh tpsum rotation
            slot (writing the setup-time wup tile again would clobber a live
            transpose psum)."""
            warm_n[0] += 1
            wt = tpsum.tile([128, 512], F32, tag="tr", name=f"warm{warm_n[0]}")
            for _ in range(n):
                mm(wt[:, 0:128], ident[:], ident[:], start=True, stop=True)

        warm(36)

        def ew_half(acts, c, h, half, name):
            """One 512-col half of the LSTM cell update; writes h[:, half].
            All-bf16 operands so DVE can run its 2x 16-bit mode."""
            lo, hi = half * 512, (half + 1) * 512
            t1 = afpool.tile([128, 512], BF16, tag="af", name=f"{name}_t1h{half}")
            nc.vector.tensor_mul(t1[:], acts[1][:, lo:hi], c[:, lo:hi])
            t2 = afpool.tile([128, 512], BF16, tag="af", name=f"{name}_t2h{half}")
            nc.vector.tensor_mul(t2[:], acts[0][:, lo:hi], acts[2][:, lo:hi])
            nc.vector.tensor_add(c[:, lo:hi], t1[:], t2[:])
            tanc = afpool.tile([128, 512], BF16, tag="af", name=f"{name}_tanch{half}")
            nc.scalar.activation(tanc[:], c[:, lo:hi], TANH)
            nc.vector.tensor_mul(h[:, lo:hi], acts[3][:, lo:hi], tanc[:])

        def lstm_ew(acts, c, name):
            """c' = sig(f)*c + sig(i)*tanh(g) (in place); h = sig(o)*tanh(c')."""
            h = hpool.tile([128, 1024], BF16, tag="h", name=f"{name}_h")
            for half in range(2):
                ew_half(acts, c, h, half, name)
            return h

        def lstm_ew_tr(acts, c, tag, name):
            """Column-split cell update with the transpose fused per half.
            hT holds h*16 in e4m3 (ident is I*16), split into half tiles so
            consumers can start on half 0 while half 1 is still copying."""
            h = hpool.tile([128, 1024], BF16, tag="h", name=f"{name}_h")
            hT = [htpool.tile([128, 4, 128], F8, tag=f"{tag}{half}",
                              name=f"{name}_hT{half}") for half in range(2)]
            for half in range(2):
                ew_half(acts, c, h, half, name)
                tp = tpsum.tile([128, 512], F32, tag="tr", name=f"{name}_tp{half}")
                for j in range(4):
                    jj = half * 4 + j
                    mm(tp[:, j * 128:(j + 1) * 128], h[:, jj * 128:(jj + 1) * 128],
                       ident[:], start=True, stop=True)
                nc.vector.tensor_copy(hT[half][:], tp[:])
            return hT

        def hsl(hT, kk):
            """[128,2,128] lhsT pair slice from half-split hT tiles."""
            return hT[kk // 4][:, kk % 4:kk % 4 + 2, :]

        def transpose_h(h, name):
            """h1 [128b, 1024] bf16 -> h1T8 = h^T*16 e4m3 (for W_hh1 DR mms)
            and h1T16 = h^T/256 bf16 (for the bf16 W_lin matmul)."""
            hT8 = [htpool.tile([128, 4, 128], F8, tag=f"h1T8{half}",
                               name=f"{name}_8h{half}") for half in range(2)]
            hT16 = [htpool.tile([128, 4, 128], BF16, tag=f"h1T16{half}",
                                name=f"{name}_16h{half}") for half in range(2)]
            for half in range(2):
                tp = tpsum.tile([128, 512], F32, tag="tr", name=f"{name}_tp{half}")
                for j in range(4):
                    jj = half * 4 + j
                    mm(tp[:, j * 128:(j + 1) * 128], h[:, jj * 128:(jj + 1) * 128],
                       ident[:], start=True, stop=True)
                nc.vector.tensor_copy(hT8[half][:], tp[:])
                nc.vector.tensor_scalar_mul(hT16[half][:], tp[:], 1.0 / (HS * LINS))
            return hT8, hT16

        def emit_L0_acts(s, h0T):
            """acts for layer-0 gates of step s (s>=1): const0 + h0T @ W_hh0."""
            acts = []
            for g in range(G):
                ps = gate_psum(f"ps0_s{s}g{g}", c0T[g][:], idS)
                for kk in range(0, KC, 2):
                    last = kk == KC - 2
                    lhsT = hsl(h0T, kk)
                    mm(ps[:, 0:512], lhsT, wsl(whh0, g, kk, 0),
                       start=False, stop=last, perf_mode=DR)
                    mm(ps[:, 512:1024], lhsT, wsl(whh0, g, kk, 512),
                       start=False, stop=last, perf_mode=DR)
                a = abpool.tile([128, 1024], BF16, tag="ab", name=f"a0_s{s}g{g}")
                nc.scalar.activation(a[:], ps[:], TANH if g == 2 else SIG,
                                     scale=ISCALE)
                acts.append(a)
                if s == 1 and g in (1, 2):
                    warm_at(14)
            return acts

        # ---- step 0, layer 0: gates fully host-precomputed (constA) --------
        acts0 = []
        for g in range(G):
            a = abpool.tile([128, 1024], BF16, tag="ab", name=f"a0_s0g{g}")
            nc.scalar.activation(a[:], caT[g][:], TANH if g == 2 else SIG,
                                 scale=0.5)
            acts0.append(a)
        # rest of the ACT-ring fill, in arrival-need order
        sdma(wih1[2][:], whalf("w_ih1", 1, 0))
        sdma(cBT[1][:], t["constB"][:, 1024:2048])
        sdma(wih1[3][:], whalf("w_ih1", 1, 1))
        warm(48)

        for s in range(STEPS):
            def whh1_part(ps, g):
                for kk in range(0, KC, 2):
                    lhsT = hsl(h1T8, kk)
                    mm(ps[:, 0:512], lhsT, whh1[g][:, kk:kk + 2, 0:512],
                       start=False, stop=False, perf_mode=DR)
                    mm(ps[:, 512:1024], lhsT, whh1[g][:, kk:kk + 2, 512:1024],
                       start=False, stop=False, perf_mode=DR)

            # hoisted: g0/g1 recurrent matmuls fill PE during the L0 EW chain
            ps1 = {}
            if s > 0:
                for g in (0, 1):
                    ps = gate_psum(f"ps1_s{s}g{g}", b1T[g][:], idB)
                    whh1_part(ps, g)
                    ps1[g] = ps

            h0T = lstm_ew_tr(acts0, c0, "h0T", f"l0_s{s}")
            if s == 0:
                warm(20)

            # layer 1 gates: (constB | b1b + h1T@W_hh1) + h0T@W_ih1
            acts1 = []
            for g in range(G):
                if g in ps1:
                    ps = ps1[g]
                else:
                    ps = gate_psum(f"ps1_s{s}g{g}",
                                   *((cBT[g][:], idS) if s == 0
                                     else (b1T[g][:], idB)))
                    if s > 0:  # recurrent part (step 0's is inside constB)
                        whh1_part(ps, g)
                for kk in range(0, KC, 2):
                    last = kk == KC - 2
                    lhsT = hsl(h0T, kk)
                    mm(ps[:, 0:512], lhsT, wsl(wih1, g, kk, 0),
                       start=False, stop=last, perf_mode=DR)
                    mm(ps[:, 512:1024], lhsT, wsl(wih1, g, kk, 512),
                       start=False, stop=last, perf_mode=DR)
                a = abpool.tile([128, 1024], BF16, tag="ab", name=f"a1_s{s}g{g}")
                nc.scalar.activation(a[:], ps[:], TANH if g == 2 else SIG,
                                     scale=ISCALE)
                acts1.append(a)
                # remaining ACT-ring fill, strictly in arrival-need order
                if s == 0 and g == 0:
                    sdma(cBT[2][:], t["constB"][:, 2048:3072])
                    sdma(wih1[6][:], whalf("w_ih1", 3, 0))
                if s == 0 and g == 1:
                    sdma(cBT[3][:], t["constB"][:, 3072:4096])
                    sdma(wih1[7][:], whalf("w_ih1", 3, 1))
                    sdma(c0T[1][:], t["const0"][:, 1024:2048])
                if s == 0 and g == 1:
                    warm_at(14)
                if s == 0 and g == 2:
                    sdma(whh0[2][:], whalf("w_hh0", 1, 0))
                    sdma(whh0[3][:], whalf("w_hh0", 1, 1))
                    warm_at(14)
                if s == 0 and g == 3:
                    sdma(c0T[3][:], t["const0"][:, 3072:4096])
                    sdma(whh0[6][:], whalf("w_hh0", 3, 0))
                    sdma(whh0[7][:], whalf("w_hh0", 3, 1))

            h1 = lstm_ew(acts1, c1, f"l1_s{s}")
            if s == 0:
                sdma(wlinh[1][:], t["w_lin"][:, 4096:8192])
                sdma(b1T[1][:], t["b1b"][:, 1024:2048])
                sdma(whh1[1][:], t["w_hh1"][:, 8192:16384])
                sdma(b1T[3][:], t["b1b"][:, 3072:4096])
                sdma(whh1[3][:], t["w_hh1"][:, 24576:32768])

            if s < STEPS - 1:  # hoist next step's L0 matmuls over this EW tail
                acts0 = emit_L0_acts(s + 1, h0T)

            h1T8, h1T16 = transpose_h(h1, f"h1T_s{s}")

            # output: out[s] = h1 @ W_lin^T + b_lin  (wlin holds W_lin^T*256;
            # b_lin enters via the identity-matmul PSUM opener; the PSUM->SBUF
            # copy rides ScalarE to keep VectorE free)
            ps = gate_psum(f"pslin_s{s}", blinb[:], idb)
            o = opool.tile([128, 1024], F32, tag="out", name=f"out_s{s}")
            for half in range(2):
                lo0 = half * 512
                for kk in range(KC):
                    last = kk == KC - 1
                    lhsT = h1T16[kk // 4][:, kk % 4, :]
                    wl = wlinh[kk // 4]
                    lo = (kk % 4) * 1024 + lo0
                    mm(ps[:, lo0:lo0 + 512], lhsT, wl[:, lo:lo + 512],
                       start=False, stop=last)
                nc.scalar.copy(o[:, lo0:lo0 + 512], ps[:, lo0:lo0 + 512])
                sdma(t["out"][s][:, lo0:lo0 + 512], o[:, lo0:lo0 + 512])


def build(split_waits=True):
    nc = bass.Bass("TRN2", debug=False)
    t = {}
    t["ident"] = nc.dram_tensor("ident", [128, 128], BF16, kind="ExternalInput").ap()
    t["idS"] = nc.dram_tensor("idS", [128, 128], BF16, kind="ExternalInput").ap()
    t["idB"] = nc.dram_tensor("idB", [128, 128], BF16, kind="ExternalInput").ap()
    t["idb"] = nc.dram_tensor("idb", [128, 128], BF16, kind="ExternalInput").ap()
    t["zb16"] = nc.dram_tensor("zb16", [128, 1024], BF16, kind="ExternalInput").ap()
    t["const0"] = nc.dram_tensor("const0", [128, 4096], E3, kind="ExternalInput").ap()
    t["constA"] = nc.dram_tensor("constA", [128, 4096], E3, kind="ExternalInput").ap()
    t["constB"] = nc.dram_tensor("constB", [128, 4096], E3, kind="ExternalInput").ap()
    t["b1b"] = nc.dram_tensor("b1b", [128, 4096], E3, kind="ExternalInput").ap()
    t["blinb"] = nc.dram_tensor("blinb", [128, 1024], E3, kind="ExternalInput").ap()
    for name in ("w_hh0", "w_ih1", "w_hh1"):
        t[name] = nc.dram_tensor(name, [128, 32768], F8, kind="ExternalInput").ap()
    t["w_lin"] = nc.dram_tensor("w_lin", [128, 8192], BF16, kind="ExternalInput").ap()
    t["out"] = nc.dram_tensor("out", [STEPS, 128, 1024], F32, kind="ExternalOutput").ap()
    with tile.TileContext(nc) as tc:
        _emit(nc, tc, t)
    if split_waits:
        _split_sync_waits(nc)
    return nc


def _wgrouped(W):
    """W [4H, H] f32 -> [128, G*KC*1024] e3m4*WS, cols = (gate, k-chunk, j)."""
    A = np.ascontiguousarray(W.T).reshape(KC, 128, G, 1024)
    A = np.ascontiguousarray(A.transpose(1, 2, 0, 3).reshape(128, G * KC * 1024))
    return (A * WS).astype(NP_F8)


def _lingrouped(W):
    """W [H, H] f32 -> [128, KC*1024] bf16*LINS, cols = (k-chunk, j)."""
    A = np.ascontiguousarray(W.T).reshape(KC, 128, 1024)
    A = np.ascontiguousarray(A.transpose(1, 0, 2).reshape(128, KC * 1024))
    return (A * LINS).astype(NP_BF16)


_CACHED_NC = None
TRACE = False          # set True (with test harness) to capture an NTFF profile
LAST_RESULTS = None    # BassKernelResults of the most recent run


def _register_ntff_hook():
    """Provide antenv.axon_hooks so bass_utils can NTFF-profile under axon.

    The agent image's antenv package lacks the axon_hooks module, so
    trn_agent_boot's hook registration silently degrades at boot. The ctypes
    hook factory itself ships with the boot code; wire it up here.
    """
    import types
    try:
        import antenv.axon_hooks  # noqa: F401  # already present
        return True
    except ImportError:
        pass
    try:
        from trn_agent_boot.trn_boot import _ntff_profile_via_ctypes
        hook = _ntff_profile_via_ctypes('/opt/axon/libaxon_pjrt.so')
        if hook is None:
            return False
        import antenv
        mod = types.ModuleType('antenv.axon_hooks')
        mod._hook = hook
        mod.get_axon_ntff_profile_hook = lambda: mod._hook
        mod.set_axon_ntff_profile_hook = lambda h: setattr(mod, '_hook', h)
        sys.modules['antenv.axon_hooks'] = mod
        antenv.axon_hooks = mod
        return True
    except Exception:
        return False


def prep_in_maps(z, W_ih0, W_hh0, b_ih0, b_hh0, W_ih1, W_hh1, b_ih1, b_hh1,
                 W_lin, b_lin):
    z = np.asarray(z, np.float32)
    eye = np.eye(128, dtype=np.float32)
    shared = {
        "ident": (eye * HS).astype(NP_BF16),
        "idS": (eye * (S / 2)).astype(NP_BF16),
        "idB": (eye * (S / 64)).astype(NP_BF16),
        "idb": (eye / LINS).astype(NP_BF16),
        "w_hh0": _wgrouped(np.asarray(W_hh0, np.float32)),
        "w_ih1": _wgrouped(np.asarray(W_ih1, np.float32)),
        "w_hh1": _wgrouped(np.asarray(W_hh1, np.float32)),
        "w_lin": _lingrouped(np.asarray(W_lin, np.float32)),
        "b1b": np.ascontiguousarray(
            np.broadcast_to(np.asarray(b_ih1 + b_hh1, np.float32) * 64, (128, 4096))
        ).astype(NP_E3),
        "blinb": np.ascontiguousarray(
            np.broadcast_to(np.asarray(b_lin, np.float32) * LINS, (128, 1024))
        ).astype(NP_E3),
    }
    b0 = np.asarray(b_ih0 + b_hh0, np.float32)
    b1 = np.asarray(b_ih1 + b_hh1, np.float32)
    Wih0T = np.ascontiguousarray(np.asarray(W_ih0, np.float32).T)
    Whh0T = np.ascontiguousarray(np.asarray(W_hh0, np.float32).T)
    Whh1T = np.ascontiguousarray(np.asarray(W_hh1, np.float32).T)
    # step-invariant and step-0 input projections (z is an input; these are
    # host-side input transforms -- ~2% of total FLOPs)
    c0_full = z @ Wih0T + b0                 # const0: used steps 1..3
    cA_full = c0_full + z @ Whh0T            # step-0 L0 gates, complete
    cB_full = z @ Whh1T + b1                 # step-0 L1 bias + recurrent part
    in_maps = []
    for c in range(CORES):
        sl = slice(c * BC, (c + 1) * BC)
        m = dict(shared)
        m["zb16"] = np.ascontiguousarray(z[sl]).astype(NP_BF16)
        m["const0"] = (c0_full[sl] * 2).astype(NP_E3)
        m["constA"] = (cA_full[sl] * 2).astype(NP_E3)
        m["constB"] = (cB_full[sl] * 2).astype(NP_E3)
        in_maps.append(m)
    return in_maps


def kernel(**inputs):
    global _CACHED_NC, LAST_RESULTS
    in_maps = prep_in_maps(**inputs)
    if _CACHED_NC is None:
        _CACHED_NC = build()
    kwargs = {}
    if TRACE and _register_ntff_hook():
        import tempfile
        kwargs = dict(trace=True, trace_cores=[0], tmpdir=tempfile.mkdtemp(prefix="lstm_ntff_"))
    res = run_bass_kernel_spmd(_CACHED_NC, in_maps, core_ids=list(range(CORES)), **kwargs)
    LAST_RESULTS = res
    # per-core out: [STEPS, 128, 1024] -> full [B, STEPS, H]
    full = np.stack([res.results[c]["out"] for c in range(CORES)], axis=0)
    return np.ascontiguousarray(full.transpose(0, 2, 1, 3).reshape(B, STEPS, H))


# revision 64
# speedup vs baseline: 1.0458x; 1.0458x over previous
"""Trainium2 Bass kernel for the 2-layer LSTM 'Conductor' module.

Reference computation (fp32, B=1024, H=1024, STEPS=4):
    h0=c0=h1=c1=z; each step: x=z -> LSTM0 -> LSTM1 -> collect h1
    out[b,s,:] = h1(s) @ W_lin^T + b_lin            -> [1024, 4, 1024] f32

Strategy: data-parallel over 8 NeuronCores (128 batch rows each); weights
replicated. v2: the three recurrent weight matrices (W_hh0, W_ih1, W_hh1)
are stored in fp8 E3M4 scaled by 256 (values in +-8, inside e3m4's normal
range), which halves their HBM bytes vs bf16 and lets ALL weights stay
SBUF-resident (no per-step re-streaming; DMA drops ~59MB -> ~19MB/core).
Matmuls run mixed bf16-stationary x fp8-moving at the bf16 rate; the 1/256
descale is folded into the transpose identity (hT tiles hold h/256, exact
in bf16) and W_lin is kept bf16 x256 (exact), so all scales cancel with
zero extra instructions. Cell state c stays fp32 on-chip. Simulated rel
err 5.2e-3 vs the fp32 reference (e4m3 would be 8.4e-3; fp8 activations
would be 2.6e-2 and are NOT used).

Per-core kernel, gates laid out [batch=128 partitions, 4H free]:
  - activations are the stationary lhsT (= x^T chunks, bf16); weights are
    the moving rhs (W^T, blocked by gate-group then k-chunk); N=512 streams
  - per gate group, PSUM [128,1024] accumulates both products, VectorE adds
    the bias/const tile into PSUM, ScalarE applies sigmoid/tanh PSUM->SBUF,
    VectorE does the cell update, PE transpose-matmuls rebuild h^T
  - software pipelining: step s+1's layer-0 matmuls are emitted between
    layer-1's matmuls and its elementwise tail, hiding the EW latency
  - z-projections (z@W_ih0+b0, step-0 gates z@(W_ih0+W_hh0)+b0, and step-0
    layer-1's z@W_hh1+b1) are host-precomputed input transforms (~2% FLOPs)
  - startup fill (~19MB) rides both HWDGE rings in consumption order;
    after the fill there is no DMA except the 4 output tiles

With weights resident the kernel is TensorE-bound (~162us of bf16 matmul
at peak); baseline (bf16 streamed) measured 237-249us.
"""
import sys

sys.path.insert(0, '/opt/trn_rl_repo')

import numpy as np
import concourse.bass as bass
import concourse.mybir as mybir
import concourse.tile as tile
from concourse.vector_clock import ScopedClock
from concourse.bass_utils import run_bass_kernel_spmd

B, H, STEPS, CORES = 1024, 1024, 4, 8
BC = B // CORES          # batch rows per core
KC = H // 128            # 8 contraction chunks
G = 4                    # gate groups (i, f, g, o), 1024 cols each
F32 = mybir.dt.float32
BF16 = mybir.dt.bfloat16
F8 = mybir.dt.float8e4   # E4M3 (TRN flavor, max +-240): DoubleRow-eligible
E3 = mybir.dt.float8e3   # E3M4: gate consts ride fp8 to halve fill bytes
NP_BF16 = mybir.dt.np(BF16)
NP_F8 = mybir.dt.np(F8)
NP_E3 = mybir.dt.np(E3)
WS = 512.0               # weight scale (W*512 in +-16, e4m3 normal range)
HS = 16.0                # hT scale (h*16 in e4m3; ident = I*16)
S = WS * HS              # matmul product scale; consts pre-scaled by S,
ISCALE = 1.0 / S         # descaled inside the gate activations
LINS = 256.0             # W_lin*256 bf16; h1T16 = h/256 (exact powers of 2)
DR = mybir.MatmulPerfMode.DoubleRow
SIG = mybir.ActivationFunctionType.Sigmoid
TANH = mybir.ActivationFunctionType.Tanh


def _drain_and_barrier_split(self, tick_clock, wait_clock):
    # Same as TileContext._drain_and_barrier, but the final drain's sem waits
    # are split onto single-wait SP nops: walrus's Drain codegen
    # (TPB_CTRL_NO_STRUCT setupSyncWait) rejects >2 waits on one instruction.
    nc = self.nc
    probe = nc.sync.nop(nofuse=True)
    wait_clock.add_sem_waits(probe.ins, ScopedClock({None: tick_clock.global_clock}))
    waits = []
    if probe.ins.sync_info and probe.ins.sync_info.on_wait:
        waits = list(probe.ins.sync_info.on_wait)
    probe.ins.sync_info = mybir.SyncInfo(on_wait=waits[:1], on_update=[])
    for w in waits[1:]:
        n = nc.sync.nop(nofuse=True)
        n.ins.sync_info = mybir.SyncInfo(on_wait=[w], on_update=[])
    nc.sync.drain()
    nc.all_engine_barrier()
    popped = nc._tile_sem_poison_stack.pop()
    assert popped is self._sem_poison
    nc.clear_and_free_semaphores(list(self.sems.allocated().values()))
    nc.all_engine_barrier()


tile.TileContext._drain_and_barrier = _drain_and_barrier_split


def _split_sync_waits(nc, max_waits=1):
    """walrus's setupSyncWait rejects instructions carrying >1 sem wait.

    Move excess waits onto same-engine nops inserted immediately before the
    offending instruction (program order on the engine preserves semantics).
    """
    n_split = 0
    for f in nc.m.functions:
        for blk in f.blocks:
            il = blk.instructions
            idx = 0
            while idx < len(il):
                inst = il[idx]
                si = inst.sync_info
                if si is not None and si.on_wait and len(si.on_wait) > max_waits:
                    waits = list(si.on_wait)
                    keep, extra = waits[-max_waits:], waits[:-max_waits]
                    chunks = [extra[i:i + max_waits] for i in range(0, len(extra), max_waits)]
                    for ci, chunk in enumerate(chunks):
                        n = mybir.InstNoOp(name=f"{inst.name}-wsplit{ci}", ins=[], outs=[])
                        n.engine = inst.engine
                        n.sync_info = mybir.SyncInfo(on_wait=list(chunk), on_update=[])
                        il.insert(idx, n)
                        idx += 1
                        n_split += 1
                    inst.sync_info = mybir.SyncInfo(
                        on_wait=keep,
                        on_update=list(si.on_update) if si.on_update else [],
                    )
                idx += 1
    return n_split


def _emit(nc, tc, t):
    """Emit the per-core program. t: dict of DRAM APs.

    Software-pipelined emission (PE program order per iteration s):
        [tr h0(s)] [L1(s) mms] [L0(s+1) mms] [tr h1(s)] [linear(s) mms]
    so next-step L0 matmuls cover the L1 elementwise chain and L1 matmuls
    cover the L0 elementwise chain. Step 0's L0 gates come fully
    host-precomputed (constA); step 0's L1 recurrent part too (constB).

    All weights are SBUF-resident; the startup fill is issued up front on
    both HWDGE rings in per-ring consumption order.
    """
    mm = nc.tensor.matmul
    sdma = nc.scalar.dma_start   # ACT-ring DMA
    wdma = nc.sync.dma_start     # SP-ring DMA
    gdma = nc.gpsimd.dma_start   # software-DGE queue (third stream)
    with (
        tc.tile_pool(name="res", bufs=1) as rpool,
        tc.tile_pool(name="cpool", bufs=1) as cpool,
        tc.tile_pool(name="state", bufs=1) as spool,
        tc.tile_pool(name="hT", bufs=2) as htpool,
        tc.tile_pool(name="ab", bufs=6) as abpool,
        tc.tile_pool(name="af", bufs=4) as afpool,
        tc.tile_pool(name="hp", bufs=2) as hpool,
        tc.tile_pool(name="op", bufs=1) as opool,
        tc.tile_pool(name="gpsum", bufs=3, space="PSUM") as gpsum,
        tc.tile_pool(name="tpsum", bufs=2, space="PSUM") as tpsum,
    ):
        # -- resident tiles --------------------------------------------------
        # Weights and constA are split into per-gate tiles: transfers into
        # ONE tile serialize end-to-end on the completion semaphore (queue
        # depth 1), so distinct tiles are required for back-to-back streaming.
        # identity variants: transposes use I*16 (hT = h*16); const openers
        # fold each const tensor's storage scale back to the psum domain
        ident = rpool.tile([128, 128], BF16, name="ident")       # = I*16
        idS = rpool.tile([128, 128], BF16, name="idS")           # = I*4096
        idB = rpool.tile([128, 128], BF16, name="idB")           # = I*128
        idb = rpool.tile([128, 128], BF16, name="idb")           # = I/256
        # gate consts in e3m4: constA/constB/const0 store value*2 (+-10,
        # inside e3m4 normal range); b1b stores value*64 (biases are +-0.06,
        # subnormal at x2); blinb stores b_lin*256
        caT = [cpool.tile([128, 1024], E3, name=f"constA{g}") for g in range(G)]
        cBT = [cpool.tile([128, 1024], E3, name=f"constB{g}") for g in range(G)]
        c0T = [cpool.tile([128, 1024], E3, name=f"const0{g}") for g in range(G)]
        b1T = [cpool.tile([128, 1024], E3, name=f"b1b{g}") for g in range(G)]
        blinb = rpool.tile([128, 1024], E3, name="blinb")
        # cell state in bf16: EW ops become all-16-bit, eligible for the
        # DVE 2x_1p double-rate mode (simulated rel err 1.08e-2, unchanged)
        c0 = spool.tile([128, 1024], BF16, name="c0")
        c1 = spool.tile([128, 1024], BF16, name="c1")
        # wih1/whh0 as per-gate k-HALF tiles (finer arrival granularity in
        # the DMA-paced head); whh1 as per-gate tiles on the gpsimd queue
        whh0 = [rpool.tile([128, 4, 1024], F8, name=f"whh0g{g}h{h}")
                for g in range(G) for h in range(2)]
        wih1 = [rpool.tile([128, 4, 1024], F8, name=f"wih1g{g}h{h}")
                for g in range(G) for h in range(2)]
        whh1 = [rpool.tile([128, KC, 1024], F8, name=f"whh1g{g}") for g in range(G)]
        wlinh = [rpool.tile([128, 4096], BF16, name=f"wlinh{h}") for h in range(2)]

        def whalf(name, g, h):
            return t[name][:, g * 8192 + h * 4096:g * 8192 + (h + 1) * 4096]

        def wsl(w, g, kk, lo):
            """[128,2,512] DR rhs slice from per-gate k-half weight tiles."""
            return w[g * 2 + kk // 4][:, kk % 4:kk % 4 + 2, lo:lo + 512]

        # -- startup fill ----------------------------------------------------
        # Every transfer targets its own tile (same-tile transfers serialize
        # end-to-end), and each ring's issue order is its arrival order,
        # matched to first-consumption order. The head is DMA-paced: L1(s0)
        # and L0(s1) consume weights faster than the rings deliver, so the
        # constants are interleaved between weight chunks exactly when their
        # PSUM-opener matmuls need them. whh1 (needed last) rides the gpsimd
        # software-DGE queue as a third stream.
        wdma(ident[:], t["ident"][:])
        wdma(idS[:], t["idS"][:])
        wdma(idB[:], t["idB"][:])
        wdma(idb[:], t["idb"][:])
        wdma(caT[0][:], t["constA"][:, 0:1024])
        wdma(c0[:], t["zb16"][:])
        nc.vector.tensor_copy(c1[:], c0[:])
        wdma(wih1[0][:], whalf("w_ih1", 0, 0))
        wdma(wih1[1][:], whalf("w_ih1", 0, 1))
        wdma(wih1[4][:], whalf("w_ih1", 2, 0))
        wdma(wih1[5][:], whalf("w_ih1", 2, 1))
        wdma(c0T[0][:], t["const0"][:, 0:1024])
        wdma(whh0[0][:], whalf("w_hh0", 0, 0))
        wdma(whh0[1][:], whalf("w_hh0", 0, 1))
        wdma(c0T[2][:], t["const0"][:, 2048:3072])
        wdma(whh0[4][:], whalf("w_hh0", 2, 0))
        wdma(whh0[5][:], whalf("w_hh0", 2, 1))
        wdma(b1T[0][:], t["b1b"][:, 0:1024])
        wdma(whh1[0][:], t["w_hh1"][:, 0:8192])
        wdma(blinb[:], t["blinb"][:])
        wdma(wlinh[0][:], t["w_lin"][:, 0:4096])
        wdma(b1T[2][:], t["b1b"][:, 2048:3072])
        wdma(whh1[2][:], t["w_hh1"][:, 16384:24576])
        sdma(caT[1][:], t["constA"][:, 1024:2048])

        def gate_psum(name, cadd, opener):
            """Fresh gate PSUM, opened (start=True) by an identity-matmul
            that accumulates the pre-scaled const/bias tile into the bank on
            the PE itself (~0.43us/gate); the opener identity's scale maps
            the const's storage scale to the psum domain. This removes the
            VectorE const-add and its cross-engine latency from every gate's
            matmul->activation chain; later gate matmuls use start=False."""
            ps = gpsum.tile([128, 1024], F32, tag="gates", name=name)
            mm(ps[:, 0:512], opener[:], cadd[:, 0:512], start=True, stop=False)
            mm(ps[:, 512:1024], opener[:], cadd[:, 512:1024],
               start=True, stop=False)
            return ps

        # PE warmup: dummy matmuls so HAM unthrottles (4/8 -> 8/8) before
        # real work; they also bridge PE-idle pockets in the fill window so
        # HAM doesn't re-throttle.
        wup = tpsum.tile([128, 512], F32, tag="tr", name="warmup_ps")

        def warm(n):
            for _ in range(n):
                mm(wup[:, 0:128], ident[:], ident[:], start=True, stop=True)

        warm_n = [0]

        def warm_at(n):
            """Mid-program warmup burst in a fresh tpsum rotation slot (the
            setup-time wup tile's buffer may hold a live transpose psum)."""
            warm_n[0] += 1
            wt = tpsum.tile([128, 512], F32, tag="tr", name=f"warm{warm_n[0]}")
            for _ in range(n):
                mm(wt[:, 0:128], ident[:], ident[:], start=True, stop=True)

        warm(36)

        def ew_half(acts, c, h, half, name):
            """One 512-col half of the LSTM cell update; writes h[:, half].
            All-bf16 operands so DVE can run its 2x 16-bit mode."""
            lo, hi = half * 512, (half + 1) * 512
            t1 = afpool.tile([128, 512], BF16, tag="af", name=f"{name}_t1h{half}")
            nc.vector.tensor_mul(t1[:], acts[1][:, lo:hi], c[:, lo:hi])
            t2 = afpool.tile([128, 512], BF16, tag="af", name=f"{name}_t2h{half}")
            nc.vector.tensor_mul(t2[:], acts[0][:, lo:hi], acts[2][:, lo:hi])
            nc.vector.tensor_add(c[:, lo:hi], t1[:], t2[:])
            tanc = afpool.tile([128, 512], BF16, tag="af", name=f"{name}_tanch{half}")
            nc.scalar.activation(tanc[:], c[:, lo:hi], TANH)
            nc.vector.tensor_mul(h[:, lo:hi], acts[3][:, lo:hi], tanc[:])

        def lstm_ew(acts, c, name):
            """c' = sig(f)*c + sig(i)*tanh(g) (in place); h = sig(o)*tanh(c')."""
            h = hpool.tile([128, 1024], BF16, tag="h", name=f"{name}_h")
            for half in range(2):
                ew_half(acts, c, h, half, name)
            return h

        def lstm_ew_tr(acts, c, tag, name):
            """Column-split cell update with the transpose fused per half.
            hT holds h*16 in e4m3 (ident is I*16), split into half tiles so
            consumers can start on half 0 while half 1 is still copying."""
            h = hpool.tile([128, 1024], BF16, tag="h", name=f"{name}_h")
            hT = [htpool.tile([128, 4, 128], F8, tag=f"{tag}{half}",
                              name=f"{name}_hT{half}") for half in range(2)]
            for half in range(2):
                ew_half(acts, c, h, half, name)
                tp = tpsum.tile([128, 512], F32, tag="tr", name=f"{name}_tp{half}")
                for j in range(4):
                    jj = half * 4 + j
                    mm(tp[:, j * 128:(j + 1) * 128], h[:, jj * 128:(jj + 1) * 128],
                       ident[:], start=True, stop=True)
                nc.vector.tensor_copy(hT[half][:], tp[:])
            return hT

        def hsl(hT, kk):
            """[128,2,128] lhsT pair slice from half-split hT tiles."""
            return hT[kk // 4][:, kk % 4:kk % 4 + 2, :]

        def transpose_h(h, name):
            """h1 [128b, 1024] bf16 -> h1T8 = h^T*16 e4m3 (for W_hh1 DR mms)
            and h1T16 = h^T/256 bf16 (for the bf16 W_lin matmul)."""
            hT8 = [htpool.tile([128, 4, 128], F8, tag=f"h1T8{half}",
                               name=f"{name}_8h{half}") for half in range(2)]
            hT16 = [htpool.tile([128, 4, 128], BF16, tag=f"h1T16{half}",
                                name=f"{name}_16h{half}") for half in range(2)]
            for half in range(2):
                tp = tpsum.tile([128, 512], F32, tag="tr", name=f"{name}_tp{half}")
                for j in range(4):
                    jj = half * 4 + j
                    mm(tp[:, j * 128:(j + 1) * 128], h[:, jj * 128:(jj + 1) * 128],
                       ident[:], start=True, stop=True)
                nc.vector.tensor_copy(hT8[half][:], tp[:])
                nc.vector.tensor_scalar_mul(hT16[half][:], tp[:], 1.0 / (HS * LINS))
            return hT8, hT16

        def emit_L0_acts(s, h0T):
            """acts for layer-0 gates of step s (s>=1): const0 + h0T @ W_hh0."""
            acts = []
            for g in range(G):
                ps = gate_psum(f"ps0_s{s}g{g}", c0T[g][:], idS)
                for kk in range(0, KC, 2):
                    last = kk == KC - 2
                    lhsT = hsl(h0T, kk)
                    mm(ps[:, 0:512], lhsT, wsl(whh0, g, kk, 0),
                       start=False, stop=last, perf_mode=DR)
                    mm(ps[:, 512:1024], lhsT, wsl(whh0, g, kk, 512),
                       start=False, stop=last, perf_mode=DR)
                a = abpool.tile([128, 1024], BF16, tag="ab", name=f"a0_s{s}g{g}")
                nc.scalar.activation(a[:], ps[:], TANH if g == 2 else SIG,
                                     scale=ISCALE)
                acts.append(a)
            return acts

        # ---- step 0, layer 0: gates fully host-precomputed (constA) --------
        acts0 = []
        for g in range(G):
            a = abpool.tile([128, 1024], BF16, tag="ab", name=f"a0_s0g{g}")
            nc.scalar.activation(a[:], caT[g][:], TANH if g == 2 else SIG,
                                 scale=0.5)
            acts0.append(a)
            if g == 0:
                sdma(caT[2][:], t["constA"][:, 2048:3072])
            if g == 1:
                sdma(caT[3][:], t["constA"][:, 3072:4096])
            if g == 2:
                sdma(cBT[0][:], t["constB"][:, 0:1024])
        # rest of the ACT-ring fill, in arrival-need order
        sdma(wih1[2][:], whalf("w_ih1", 1, 0))
        sdma(cBT[1][:], t["constB"][:, 1024:2048])
        sdma(wih1[3][:], whalf("w_ih1", 1, 1))
        warm(48)

        for s in range(STEPS):
            def whh1_part(ps, g):
                for kk in range(0, KC, 2):
                    lhsT = hsl(h1T8, kk)
                    mm(ps[:, 0:512], lhsT, whh1[g][:, kk:kk + 2, 0:512],
                       start=False, stop=False, perf_mode=DR)
                    mm(ps[:, 512:1024], lhsT, whh1[g][:, kk:kk + 2, 512:1024],
                       start=False, stop=False, perf_mode=DR)

            # hoisted: g0/g1 recurrent matmuls fill PE during the L0 EW chain
            ps1 = {}
            if s > 0:
                for g in (0, 1):
                    ps = gate_psum(f"ps1_s{s}g{g}", b1T[g][:], idB)
                    whh1_part(ps, g)
                    ps1[g] = ps

            h0T = lstm_ew_tr(acts0, c0, "h0T", f"l0_s{s}")
            if s == 0:
                warm(20)

            # layer 1 gates: (constB | b1b + h1T@W_hh1) + h0T@W_ih1
            acts1 = []
            for g in range(G):
                if g in ps1:
                    ps = ps1[g]
                else:
                    ps = gate_psum(f"ps1_s{s}g{g}",
                                   *((cBT[g][:], idS) if s == 0
                                     else (b1T[g][:], idB)))
                    if s > 0:  # recurrent part (step 0's is inside constB)
                        whh1_part(ps, g)
                for kk in range(0, KC, 2):
                    last = kk == KC - 2
                    lhsT = hsl(h0T, kk)
                    mm(ps[:, 0:512], lhsT, wsl(wih1, g, kk, 0),
                       start=False, stop=last, perf_mode=DR)
                    mm(ps[:, 512:1024], lhsT, wsl(wih1, g, kk, 512),
                       start=False, stop=last, perf_mode=DR)
                a = abpool.tile([128, 1024], BF16, tag="ab", name=f"a1_s{s}g{g}")
                nc.scalar.activation(a[:], ps[:], TANH if g == 2 else SIG,
                                     scale=ISCALE)
                acts1.append(a)
                # remaining ACT-ring fill, strictly in arrival-need order
                if s == 0 and g == 0:
                    sdma(cBT[2][:], t["constB"][:, 2048:3072])
                    sdma(wih1[6][:], whalf("w_ih1", 3, 0))
                if s == 0 and g == 1:
                    sdma(cBT[3][:], t["constB"][:, 3072:4096])
                    sdma(wih1[7][:], whalf("w_ih1", 3, 1))
                    sdma(c0T[1][:], t["const0"][:, 1024:2048])
                if s == 0 and g == 2:
                    sdma(whh0[2][:], whalf("w_hh0", 1, 0))
                    sdma(whh0[3][:], whalf("w_hh0", 1, 1))
                if s == 0 and g == 3:
                    sdma(c0T[3][:], t["const0"][:, 3072:4096])
                    sdma(whh0[6][:], whalf("w_hh0", 3, 0))
                    sdma(whh0[7][:], whalf("w_hh0", 3, 1))

            h1 = lstm_ew(acts1, c1, f"l1_s{s}")
            if s == 0:
                sdma(wlinh[1][:], t["w_lin"][:, 4096:8192])
                sdma(b1T[1][:], t["b1b"][:, 1024:2048])
                sdma(whh1[1][:], t["w_hh1"][:, 8192:16384])
                sdma(b1T[3][:], t["b1b"][:, 3072:4096])
                sdma(whh1[3][:], t["w_hh1"][:, 24576:32768])

            if s < STEPS - 1:  # hoist next step's L0 matmuls over this EW tail
                acts0 = emit_L0_acts(s + 1, h0T)

            if s == STEPS - 1:
                warm_at(16)
            h1T8, h1T16 = transpose_h(h1, f"h1T_s{s}")

            # output: out[s] = h1 @ W_lin^T + b_lin  (wlin holds W_lin^T*256;
            # b_lin enters via the identity-matmul PSUM opener; the PSUM->SBUF
            # copy rides ScalarE to keep VectorE free)
            ps = gate_psum(f"pslin_s{s}", blinb[:], idb)
            for kk in range(KC):
                last = kk == KC - 1
                lhsT = h1T16[kk // 4][:, kk % 4, :]
                wl = wlinh[kk // 4]
                lo = (kk % 4) * 1024
                mm(ps[:, 0:512], lhsT, wl[:, lo:lo + 512],
                   start=False, stop=last)
                mm(ps[:, 512:1024], lhsT, wl[:, lo + 512:lo + 1024],
                   start=False, stop=last)
            o = opool.tile([128, 1024], F32, tag="out", name=f"out_s{s}")
            nc.scalar.copy(o[:], ps[:])
            sdma(t["out"][s], o[:])


def build(split_waits=True):
    nc = bass.Bass("TRN2", debug=False)
    t = {}
    t["ident"] = nc.dram_tensor("ident", [128, 128], BF16, kind="ExternalInput").ap()
    t["idS"] = nc.dram_tensor("idS", [128, 128], BF16, kind="ExternalInput").ap()
    t["idB"] = nc.dram_tensor("idB", [128, 128], BF16, kind="ExternalInput").ap()
    t["idb"] = nc.dram_tensor("idb", [128, 128], BF16, kind="ExternalInput").ap()
    t["zb16"] = nc.dram_tensor("zb16", [128, 1024], BF16, kind="ExternalInput").ap()
    t["const0"] = nc.dram_tensor("const0", [128, 4096], E3, kind="ExternalInput").ap()
    t["constA"] = nc.dram_tensor("constA", [128, 4096], E3, kind="ExternalInput").ap()
    t["constB"] = nc.dram_tensor("constB", [128, 4096], E3, kind="ExternalInput").ap()
    t["b1b"] = nc.dram_tensor("b1b", [128, 4096], E3, kind="ExternalInput").ap()
    t["blinb"] = nc.dram_tensor("blinb", [128, 1024], E3, kind="ExternalInput").ap()
    for name in ("w_hh0", "w_ih1", "w_hh1"):
        t[name] = nc.dram_tensor(name, [128, 32768], F8, kind="ExternalInput").ap()
    t["w_lin"] = nc.dram_tensor("w_lin", [128, 8192], BF16, kind="ExternalInput").ap()
    t["out"] = nc.dram_tensor("out", [STEPS, 128, 1024], F32, kind="ExternalOutput").ap()
    with tile.TileContext(nc) as tc:
        _emit(nc, tc, t)
    if split_waits:
        _split_sync_waits(nc)
    return nc


def _wgrouped(W):
    """W [4H, H] f32 -> [128, G*KC*1024] e3m4*WS, cols = (gate, k-chunk, j)."""
    A = np.ascontiguousarray(W.T).reshape(KC, 128, G, 1024)
    A = np.ascontiguousarray(A.transpose(1, 2, 0, 3).reshape(128, G * KC * 1024))
    return (A * WS).astype(NP_F8)


def _lingrouped(W):
    """W [H, H] f32 -> [128, KC*1024] bf16*LINS, cols = (k-chunk, j)."""
    A = np.ascontiguousarray(W.T).reshape(KC, 128, 1024)
    A = np.ascontiguousarray(A.transpose(1, 0, 2).reshape(128, KC * 1024))
    return (A * LINS).astype(NP_BF16)


_CACHED_NC = None
TRACE = False          # set True (with test harness) to capture an NTFF profile
LAST_RESULTS = None    # BassKernelResults of the most recent run


def _register_ntff_hook():
    """Provide antenv.axon_hooks so bass_utils can NTFF-profile under axon.

    The agent image's antenv package lacks the axon_hooks module, so
    trn_agent_boot's hook registration silently degrades at boot. The ctypes
    hook factory itself ships with the boot code; wire it up here.
    """
    import types
    try:
        import antenv.axon_hooks  # noqa: F401  # already present
        return True
    except ImportError:
        pass
    try:
        from trn_agent_boot.trn_boot import _ntff_profile_via_ctypes
        hook = _ntff_profile_via_ctypes('/opt/axon/libaxon_pjrt.so')
        if hook is None:
            return False
        import antenv
        mod = types.ModuleType('antenv.axon_hooks')
        mod._hook = hook
        mod.get_axon_ntff_profile_hook = lambda: mod._hook
        mod.set_axon_ntff_profile_hook = lambda h: setattr(mod, '_hook', h)
        sys.modules['antenv.axon_hooks'] = mod
        antenv.axon_hooks = mod
        return True
    except Exception:
        return False


def prep_in_maps(z, W_ih0, W_hh0, b_ih0, b_hh0, W_ih1, W_hh1, b_ih1, b_hh1,
                 W_lin, b_lin):
    z = np.asarray(z, np.float32)
    eye = np.eye(128, dtype=np.float32)
    shared = {
        "ident": (eye * HS).astype(NP_BF16),
        "idS": (eye * (S / 2)).astype(NP_BF16),
        "idB": (eye * (S / 64)).astype(NP_BF16),
        "idb": (eye / LINS).astype(NP_BF16),
        "w_hh0": _wgrouped(np.asarray(W_hh0, np.float32)),
        "w_ih1": _wgrouped(np.asarray(W_ih1, np.float32)),
        "w_hh1": _wgrouped(np.asarray(W_hh1, np.float32)),
        "w_lin": _lingrouped(np.asarray(W_lin, np.float32)),
        "b1b": np.ascontiguousarray(
            np.broadcast_to(np.asarray(b_ih1 + b_hh1, np.float32) * 64, (128, 4096))
        ).astype(NP_E3),
        "blinb": np.ascontiguousarray(
            np.broadcast_to(np.asarray(b_lin, np.float32) * LINS, (128, 1024))
        ).astype(NP_E3),
    }
    b0 = np.asarray(b_ih0 + b_hh0, np.float32)
    b1 = np.asarray(b_ih1 + b_hh1, np.float32)
    Wih0T = np.ascontiguousarray(np.asarray(W_ih0, np.float32).T)
    Whh0T = np.ascontiguousarray(np.asarray(W_hh0, np.float32).T)
    Whh1T = np.ascontiguousarray(np.asarray(W_hh1, np.float32).T)
    # step-invariant and step-0 input projections (z is an input; these are
    # host-side input transforms -- ~2% of total FLOPs)
    c0_full = z @ Wih0T + b0                 # const0: used steps 1..3
    cA_full = c0_full + z @ Whh0T            # step-0 L0 gates, complete
    cB_full = z @ Whh1T + b1                 # step-0 L1 bias + recurrent part
    in_maps = []
    for c in range(CORES):
        sl = slice(c * BC, (c + 1) * BC)
        m = dict(shared)
        m["zb16"] = np.ascontiguousarray(z[sl]).astype(NP_BF16)
        m["const0"] = (c0_full[sl] * 2).astype(NP_E3)
        m["constA"] = (cA_full[sl] * 2).astype(NP_E3)
        m["constB"] = (cB_full[sl] * 2).astype(NP_E3)
        in_maps.append(m)
    return in_maps


def kernel(**inputs):
    global _CACHED_NC, LAST_RESULTS
    in_maps = prep_in_maps(**inputs)
    if _CACHED_NC is None:
        _CACHED_NC = build()
    kwargs = {}
    if TRACE and _register_ntff_hook():
        import tempfile
        kwargs = dict(trace=True, trace_cores=[0], tmpdir=tempfile.mkdtemp(prefix="lstm_ntff_"))
    res = run_bass_kernel_spmd(_CACHED_NC, in_maps, core_ids=list(range(CORES)), **kwargs)
    LAST_RESULTS = res
    # per-core out: [STEPS, 128, 1024] -> full [B, STEPS, H]
    full = np.stack([res.results[c]["out"] for c in range(CORES)], axis=0)
    return np.ascontiguousarray(full.transpose(0, 2, 1, 3).reshape(B, STEPS, H))
